# revision 5
# baseline (speedup 1.0000x reference)
"""Multi-head self-attention (B=4, S=2048, D=1024, H=16) on 8 NeuronCores.

Sharding: core c handles batch b=c//2 and head-half h0=(c%2)*8 (8 of 16 heads).
Each core computes q/k/v projections for its heads, full attention, and a
partial output projection over its 512-wide slice of the concat dim.
Host sums the two partial outputs per batch and adds bo + Wo@bv (the value
bias commutes through softmax since probabilities sum to 1).

On-chip layout (all pre-transposed on host, d-major, bf16):
  qT/kT: [t, s] per head-pair stacked on partitions -> row-packed K=64
         score matmuls (two heads concurrently in PE row groups 0-1/2-3).
  scoresT: [j(key), i(query)] so softmax denom = partition-dim sum, obtained
         free via 64 ones-columns appended to v in the ctx matmul.
  exp:   one ACT op per j-tile over a 2-bank [128,2,512] PSUM AP, scale=1/8.
"""

import numpy as np
import ml_dtypes

from contextlib import ExitStack

import concourse.bacc as bacc
import concourse.bass as bass
import concourse.mybir as mybir
import concourse.tile as tile
from concourse.bass_utils import run_bass_kernel_spmd

BF16 = ml_dtypes.bfloat16

B, S, D, H, T = 4, 2048, 1024, 16, 64
HL = 8            # heads per core
DL = HL * T       # 512: local slice of concat dim
NC = 8            # cores
NPAIR = 4         # head pairs per core
NSB = 4           # 512-wide s/i blocks
NJB = 16          # 128-wide j blocks
NKC = 8           # 128-wide contraction chunks of D

f32 = mybir.dt.float32
bf16 = mybir.dt.bfloat16

_STATE = {}


def _build():
    nc = bacc.Bacc("TRN2", target_bir_lowering=False, debug=False, num_devices=NC)

    xq = nc.dram_tensor("xq", [D, S], bf16, kind="ExternalInput").ap()
    xk = nc.dram_tensor("xk", [D, S], bf16, kind="ExternalInput").ap()
    xv = nc.dram_tensor("xv", [D, S], bf16, kind="ExternalInput").ap()
    wq = nc.dram_tensor("wq", [D, DL], bf16, kind="ExternalInput").ap()
    wk = nc.dram_tensor("wk", [D, DL], bf16, kind="ExternalInput").ap()
    wv = nc.dram_tensor("wv", [D, DL], bf16, kind="ExternalInput").ap()
    bq2 = nc.dram_tensor("bq2", [128, NPAIR], f32, kind="ExternalInput").ap()
    bk2 = nc.dram_tensor("bk2", [128, NPAIR], f32, kind="ExternalInput").ap()
    wo = nc.dram_tensor("wo", [DL, D], bf16, kind="ExternalInput").ap()
    out = nc.dram_tensor("out", [S, D], f32, kind="ExternalOutput").ap()

    with tile.TileContext(nc) as tc:
        with ExitStack() as octx:
            # Persistent tensors
            persist = octx.enter_context(tc.tile_pool(name="persist", bufs=1))
            qT = persist.tile([128, NPAIR, S], bf16, tag="qT")
            kT = persist.tile([128, NPAIR, S], bf16, tag="kT")
            # v_aug[:, h, jb, 0:64] = v[j, t]; [..., 64:128] = 1.0
            v_aug = persist.tile([128, HL, NJB, 128], bf16, tag="v_aug")
            zT = persist.tile([128, NPAIR, S], bf16, tag="zT")
            bq_sb = persist.tile([128, NPAIR], f32, tag="bq_sb")
            bk_sb = persist.tile([128, NPAIR], f32, tag="bk_sb")
            wo_sb = persist.tile([128, NPAIR, D], bf16, tag="wo_sb")

            nc.sync.dma_start(bq_sb[:], bq2[:, :])
            nc.sync.dma_start(bk_sb[:], bk2[:, :])
            for p in range(NPAIR):
                nc.sync.dma_start(wo_sb[:, p, :], wo[bass.ts(p, 128), :])

            # ones columns of v_aug (memset whole region once)
            nc.vector.memset(v_aug[:, :, :, 64:128], 1.0)

            # ---------------- Phase 1: q/k/v projections ----------------
            with ExitStack() as p1:
                xpool = p1.enter_context(tc.tile_pool(name="xp", bufs=2))
                wpool = p1.enter_context(tc.tile_pool(name="wp", bufs=2))
                ps1 = p1.enter_context(tc.tile_pool(name="ps1", bufs=2,
                                                    space="PSUM"))

                def load_x(src):
                    t = xpool.tile([128, NKC, S], bf16, tag="x")
                    for c in range(NKC):
                        nc.sync.dma_start(t[:, c, :], src[bass.ts(c, 128), :])
                    return t

                def load_w(src):
                    t = wpool.tile([128, NKC, DL], bf16, tag="w")
                    for c in range(NKC):
                        nc.sync.dma_start(t[:, c, :], src[bass.ts(c, 128), :])
                    return t

                # q and k projections -> qT/kT [t(pair-stacked), s] + bias
                for name, xsrc, wsrc, dstT, bias in (
                    ("q", xq, wq, qT, bq_sb),
                    ("k", xk, wk, kT, bk_sb),
                ):
                    xs = load_x(xsrc)
                    ws = load_w(wsrc)
                    for p in range(NPAIR):
                        for sb in range(NSB):
                            pq = ps1.tile([128, 512], f32, tag="pq")
                            for c in range(NKC):
                                nc.tensor.matmul(
                                    out=pq[:],
                                    lhsT=ws[:, c, bass.ts(p, 128)],
                                    rhs=xs[:, c, bass.ts(sb, 512)],
                                    start=(c == 0), stop=(c == NKC - 1),
                                )
                            nc.vector.tensor_scalar_add(
                                out=dstT[:, p, bass.ts(sb, 512)],
                                in0=pq[:],
                                scalar1=bias[:, p:p + 1],
                            )

                # v projection -> v_aug[:, h, jb, 0:64]
                xs = load_x(xv)
                ws = load_w(wv)
                for jb in range(NJB):
                    pv = ps1.tile([128, HL, T], f32, tag="pv")
                    for c in range(NKC):
                        nc.tensor.matmul(
                            out=pv[:],
                            lhsT=xs[:, c, bass.ts(jb, 128)],
                            rhs=ws[:, c, :],
                            start=(c == 0), stop=(c == NKC - 1),
                        )
                    nc.vector.tensor_copy(out=v_aug[:, :, jb, 0:64], in_=pv[:])

            # ---------------- Phase 2: attention ----------------
            with ExitStack() as p2:
                scp = p2.enter_context(tc.tile_pool(name="scp", bufs=2,
                                                    space="PSUM"))
                ctp = p2.enter_context(tc.tile_pool(name="ctp", bufs=2,
                                                    space="PSUM"))
                epool = p2.enter_context(tc.tile_pool(name="ep", bufs=3))

                for p in range(NPAIR):
                    for ib in range(NSB):
                        ctx_ps = ctp.tile([128, 2, 512], f32, tag="ctx")
                        for jb in range(NJB):
                            sc = scp.tile([128, 2, 512], f32, tag="sc")
                            # transposed scores, two heads row-packed (K=64)
                            nc.tensor.matmul(
                                out=sc[:, 0, :],
                                lhsT=kT[0:64, p, bass.ts(jb, 128)],
                                rhs=qT[0:64, p, bass.ts(ib, 512)],
                                start=True, stop=True,
                            )
                            nc.tensor.matmul(
                                out=sc[:, 1, :],
                                lhsT=kT[64:128, p, bass.ts(jb, 128)],
                                rhs=qT[64:128, p, bass.ts(ib, 512)],
                                start=True, stop=True,
                            )
                            et = epool.tile([128, 2, 512], bf16, tag="et")
                            nc.scalar.activation(
                                out=et[:], in_=sc[:],
                                func=mybir.ActivationFunctionType.Exp,
                                scale=0.125,
                            )
                            # ctx accumulation; cols 64:128 of v_aug are ones
                            # -> rows 64:128 of ctx_ps = softmax denominator
                            nc.tensor.matmul(
                                out=ctx_ps[:, 0, :],
                                lhsT=v_aug[:, 2 * p, jb, :],
                                rhs=et[:, 0, :],
                                start=(jb == 0), stop=(jb == NJB - 1),
                            )
                            nc.tensor.matmul(
                                out=ctx_ps[:, 1, :],
                                lhsT=v_aug[:, 2 * p + 1, jb, :],
                                rhs=et[:, 1, :],
                                start=(jb == 0), stop=(jb == NJB - 1),
                            )
                        # normalize: zT rows 0:64 (head a), 64:128 (head b).
                        # HW: only one tensor_tensor input may be PSUM, so
                        # stage the denominator rows through SBUF first.
                        den = epool.tile([128, 2, 512], f32, tag="den")
                        nc.vector.reciprocal(
                            out=den[0:64, 0, :], in_=ctx_ps[64:128, 0, :])
                        nc.vector.reciprocal(
                            out=den[64:128, 1, :], in_=ctx_ps[64:128, 1, :])
                        nc.vector.tensor_tensor(
                            out=zT[0:64, p, bass.ts(ib, 512)],
                            in0=ctx_ps[0:64, 0, :], in1=den[0:64, 0, :],
                            op=mybir.AluOpType.mult,
                        )
                        nc.vector.tensor_tensor(
                            out=zT[64:128, p, bass.ts(ib, 512)],
                            in0=ctx_ps[0:64, 1, :], in1=den[64:128, 1, :],
                            op=mybir.AluOpType.mult,
                        )

            # ---------------- Phase 3: output projection ----------------
            with ExitStack() as p3:
                ps3 = p3.enter_context(tc.tile_pool(name="ps3", bufs=4,
                                                    space="PSUM"))
                opool = p3.enter_context(tc.tile_pool(name="op", bufs=4))
                for ib in range(S // 128):
                    for e in range(2):
                        po = ps3.tile([128, 512], f32, tag="po")
                        for p in range(NPAIR):
                            nc.tensor.matmul(
                                out=po[:],
                                lhsT=zT[:, p, bass.ts(ib, 128)],
                                rhs=wo_sb[:, p, bass.ts(e, 512)],
                                start=(p == 0), stop=(p == NPAIR - 1),
                            )
                        ot = opool.tile([128, 512], f32, tag="ot")
                        nc.vector.tensor_copy(out=ot[:], in_=po[:])
                        nc.sync.dma_start(
                            out[bass.ts(ib, 128), bass.ts(e, 512)], ot[:])

    nc.compile()
    return nc


def _prep_inputs(Q, K, V, Wq, bq, Wk, bk, Wv, bv, Wo, bo):
    """Build the 8 per-core input maps (host-side shard + transpose + cast)."""
    xt = {}  # (tensor, batch) -> [NKC,128,S] bf16
    for nm, full in (("xq", Q), ("xk", K), ("xv", V)):
        for b in range(B):
            xt[(nm, b)] = np.ascontiguousarray(full[b].T).astype(BF16)

    def w_half(W, h0):
        # W [H,T,D] -> [D, HL*T] -> [NKC,128,DL]
        w = W[h0:h0 + HL]                       # [HL,T,D]
        w = w.transpose(2, 0, 1).reshape(D, DL)  # [D, HL*T]
        return np.ascontiguousarray(w).astype(BF16)

    def b_half(bias, h0):
        return np.ascontiguousarray(
            bias[h0:h0 + HL].reshape(NPAIR, 128).T).astype(np.float32)

    in_maps = []
    for c in range(NC):
        b, half = c // 2, c % 2
        h0 = half * HL
        off = half * DL
        wo_c = np.ascontiguousarray(Wo[:, off:off + DL].T).astype(BF16)
        in_maps.append({
            "xq": xt[("xq", b)], "xk": xt[("xk", b)], "xv": xt[("xv", b)],
            "wq": w_half(Wq, h0), "wk": w_half(Wk, h0), "wv": w_half(Wv, h0),
            "bq2": b_half(bq, h0), "bk2": b_half(bk, h0),
            "wo": wo_c,
        })
    return in_maps


def _combine(results, bv, Wo, bo):
    const = (Wo.astype(np.float64) @ bv.reshape(-1).astype(np.float64)
             + bo.astype(np.float64)).astype(np.float32)
    out = np.empty((B, S, D), np.float32)
    for b in range(B):
        out[b] = results[2 * b]["out"] + results[2 * b + 1]["out"] + const
    return out


def kernel(Q, K, V, Wq, bq, Wk, bk, Wv, bv, Wo, bo, _trace=False):
    args = [np.asarray(a, np.float32) for a in
            (Q, K, V, Wq, bq, Wk, bk, Wv, bv, Wo, bo)]
    Q, K, V, Wq, bq, Wk, bk, Wv, bv, Wo, bo = args
    if "nc" not in _STATE:
        _STATE["nc"] = _build()
    nc = _STATE["nc"]
    in_maps = _prep_inputs(Q, K, V, Wq, bq, Wk, bk, Wv, bv, Wo, bo)
    try:
        res = run_bass_kernel_spmd(nc, in_maps, list(range(NC)), trace=_trace)
    except ImportError:
        res = run_bass_kernel_spmd(nc, in_maps, list(range(NC)), trace=False)
    out = _combine(res.results, bv, Wo, bo)
    if _trace:
        _STATE["last_result"] = res
    return out


# revision 20
# speedup vs baseline: 1.1010x; 1.1010x over previous
"""Multi-head self-attention (B=4, S=2048, D=1024, H=16) on 8 NeuronCores.

Sharding: core c handles batch b=c//2 and head-half h0=(c%2)*8 (8 of 16 heads).
Each core computes q/k/v projections for its heads, full attention, and a
partial output projection over its 512-wide slice of the concat dim.
Host sums the two partial outputs per batch and adds bo + Wo@bv (the value
bias commutes through softmax since probabilities sum to 1).

On-chip layout (all pre-transposed on host, d-major, bf16):
  qT/kT: [t, s] per head-pair stacked on partitions -> row-packed K=64
         score matmuls (two heads concurrently in PE row groups 0-1/2-3).
  scoresT: [j(key), i(query)] so softmax denom = partition-dim sum, obtained
         free via 64 ones-columns appended to v in the ctx matmul.
  exp:   one ACT op per j-tile over a 2-bank [128,2,512] PSUM AP, scale=1/8.
"""

import numpy as np
import ml_dtypes

from contextlib import ExitStack

import concourse.bacc as bacc
import concourse.bass as bass
import concourse.mybir as mybir
import concourse.tile as tile
from concourse.bass_utils import run_bass_kernel_spmd

BF16 = ml_dtypes.bfloat16

B, S, D, H, T = 4, 2048, 1024, 16, 64
HL = 8            # heads per core
DL = HL * T       # 512: local slice of concat dim
NC = 8            # cores
NPAIR = 4         # head pairs per core
NSB = 4           # 512-wide s/i blocks
NJB = 16          # 128-wide j blocks
NKC = 8           # 128-wide contraction chunks of D

f32 = mybir.dt.float32
bf16 = mybir.dt.bfloat16

_STATE = {}


def _build():
    nc = bacc.Bacc("TRN2", target_bir_lowering=False, debug=False, num_devices=NC)

    xq = nc.dram_tensor("xq", [D, S], bf16, kind="ExternalInput").ap()
    xk = nc.dram_tensor("xk", [D, S], bf16, kind="ExternalInput").ap()
    xv = nc.dram_tensor("xv", [D, S], bf16, kind="ExternalInput").ap()
    wq = nc.dram_tensor("wq", [D, DL], bf16, kind="ExternalInput").ap()
    wk = nc.dram_tensor("wk", [D, DL], bf16, kind="ExternalInput").ap()
    wv = nc.dram_tensor("wv", [D, DL], bf16, kind="ExternalInput").ap()
    bq2 = nc.dram_tensor("bq2", [128, NPAIR], f32, kind="ExternalInput").ap()
    bk2 = nc.dram_tensor("bk2", [128, NPAIR], f32, kind="ExternalInput").ap()
    wo = nc.dram_tensor("wo", [DL, D], bf16, kind="ExternalInput").ap()
    out = nc.dram_tensor("out", [S, D], f32, kind="ExternalOutput").ap()

    with tile.TileContext(nc) as tc:
        with ExitStack() as octx:
            # Persistent tensors
            persist = octx.enter_context(tc.tile_pool(name="persist", bufs=1))
            qT = persist.tile([128, NPAIR, S], bf16, tag="qT")
            kT = persist.tile([128, NPAIR, S], bf16, tag="kT")
            # v_aug[:, h, jb, 0:64] = v[j, t]; [..., 64:128] = 1.0
            v_aug = persist.tile([128, NJB, HL, 128], bf16, tag="v_aug")
            bq_sb = persist.tile([128, NPAIR], f32, tag="bq_sb")
            bk_sb = persist.tile([128, NPAIR], f32, tag="bk_sb")

            # whole-tensor x tiles; zT reuses xv's slot after v-proj
            xpool = octx.enter_context(tc.tile_pool(name="xp", bufs=1))
            xv_t = xpool.tile([128, NKC, S], bf16, tag="xv")
            zT = xpool.tile([128, NPAIR, S], bf16, tag="xv", name="zT")
            xspool = octx.enter_context(tc.tile_pool(name="xsp", bufs=4))
            # weights: wk/wq/wv cycle 3 slots; wo reuses wk's slot later
            wpool = octx.enter_context(tc.tile_pool(name="wp", bufs=3))
            # One PSUM pool, 8 banks total:
            #   sc   [128,2,512] x2 bufs = 4 banks (scores double-buffer)
            #   ctx  [128,2,512] x1 buf  = 2 banks
            #   gemm [128,512]   x2 bufs = 2 banks (qkv proj + out proj)
            psum = octx.enter_context(tc.tile_pool(name="psum", bufs=2,
                                                   space="PSUM"))
            epool = octx.enter_context(tc.tile_pool(name="ep", bufs=7))
            opool = octx.enter_context(tc.tile_pool(name="op", bufs=3))

            # dram views [128, chunk, cols]
            xkr = xk.rearrange("(c p) s -> p c s", p=128)
            xqr = xq.rearrange("(c p) s -> p c s", p=128)
            xvr = xv.rearrange("(c p) s -> p c s", p=128)
            wkr = wk.rearrange("(c p) d -> p c d", p=128)
            wqr = wq.rearrange("(c p) d -> p c d", p=128)
            wvr = wv.rearrange("(c p) d -> p c d", p=128)

            # ones columns of v_aug (memset whole region once)
            nc.vector.memset(v_aug[:, :, :, 64:128], 1.0)
            nc.gpsimd.dma_start(bk_sb[:], bk2[:, :])
            nc.gpsimd.dma_start(bq_sb[:], bq2[:, :])

            def proj(ws, xs, dstT, bias, p, sb):
                pq = psum.tile([128, 512], f32, tag="gemm", name="pq")
                for c in range(NKC):
                    nc.tensor.matmul(
                        out=pq[:],
                        lhsT=ws[:, c, bass.ts(p, 128)],
                        rhs=xs[:, c, :],
                        start=(c == 0), stop=(c == NKC - 1),
                    )
                nc.vector.tensor_scalar_add(
                    out=dstT[:, p, bass.ts(sb, 512)],
                    in0=pq[:],
                    scalar1=bias[:, p:p + 1],
                )

            def load_xsb(r, sb, tag, eng):
                t = xspool.tile([128, NKC, 512], bf16, tag=tag, name=tag)
                eng.dma_start(t[:], r[:, :, bass.ts(sb, 512)])
                return t

            # ---- critical path to first exp: pair-0 weight columns and
            # s-block-0 inputs first, split across the two DMA queues ----
            wk_t = wpool.tile([128, NKC, DL], bf16, tag="w", name="wk_t")
            nc.gpsimd.dma_start(wk_t[:], wkr[:])
            xk_sb = [load_xsb(xkr, 0, "xks", nc.gpsimd)]
            wq_t = wpool.tile([128, NKC, DL], bf16, tag="w", name="wq_t")
            nc.sync.dma_start(wq_t[:], wqr[:])
            xq_sb = [load_xsb(xqr, 0, "xqs", nc.sync)]
            proj(wk_t, xk_sb[0], kT, bk_sb, 0, 0)
            proj(wq_t, xq_sb[0], qT, bq_sb, 0, 0)

            # rest of the k inputs (scores jb>=4 of the first block), then
            # v inputs on both queues, then remaining q inputs
            xk_sb.append(load_xsb(xkr, 1, "xks", nc.gpsimd))
            proj(wk_t, xk_sb[1], kT, bk_sb, 0, 1)
            wv_t = wpool.tile([128, NKC, DL], bf16, tag="w", name="wv_t")
            nc.sync.dma_start(wv_t[:], wvr[:])
            for c in range(NKC):
                eng = nc.gpsimd if c % 2 == 0 else nc.sync
                eng.dma_start(xv_t[:, c, :], xvr[:, c, :])
            xk_sb.append(load_xsb(xkr, 2, "xks", nc.gpsimd))
            xk_sb.append(load_xsb(xkr, 3, "xks", nc.gpsimd))
            for sb in range(1, NSB):
                xq_sb.append(load_xsb(xqr, sb, "xqs", nc.sync))
            proj(wk_t, xk_sb[2], kT, bk_sb, 0, 2)
            proj(wk_t, xk_sb[3], kT, bk_sb, 0, 3)
            proj(wq_t, xq_sb[1], qT, bq_sb, 0, 1)
            proj(wq_t, xq_sb[2], qT, bq_sb, 0, 2)
            proj(wq_t, xq_sb[3], qT, bq_sb, 0, 3)

            # ---- v projection (PE work before first ctx; scheduler hoists
            # independent score matmuls ahead as their inputs land) ----
            for jb in range(NJB):
                pv = psum.tile([128, HL, T], f32, tag="gemm")
                for c in range(NKC):
                    nc.tensor.matmul(
                        out=pv[:],
                        lhsT=xv_t[:, c, bass.ts(jb, 128)],
                        rhs=wv_t[:, c, :],
                        start=(c == 0), stop=(c == NKC - 1),
                    )
                nc.vector.tensor_copy(out=v_aug[:, jb, :, 0:64], in_=pv[:])

            def attention(p, ib):
                ctx_ps = psum.tile([128, 2, 512], f32, tag="ctx", bufs=1)
                for jb in range(NJB):
                    sc = psum.tile([128, 2, 512], f32, tag="sc")
                    nc.tensor.matmul(
                        out=sc[:, 0, :],
                        lhsT=kT[0:64, p, bass.ts(jb, 128)],
                        rhs=qT[0:64, p, bass.ts(ib, 512)],
                        start=True, stop=True,
                    )
                    nc.tensor.matmul(
                        out=sc[:, 1, :],
                        lhsT=kT[64:128, p, bass.ts(jb, 128)],
                        rhs=qT[64:128, p, bass.ts(ib, 512)],
                        start=True, stop=True,
                    )
                    et = epool.tile([128, 2, 512], bf16, tag="et", bufs=7)
                    nc.scalar.activation(
                        out=et[:], in_=sc[:],
                        func=mybir.ActivationFunctionType.Exp,
                        scale=0.125,
                    )
                    # ctx accumulation; cols 64:128 of v_aug are ones
                    # -> rows 64:128 of ctx_ps = softmax denominator
                    nc.tensor.matmul(
                        out=ctx_ps[:, 0, :],
                        lhsT=v_aug[:, jb, 2 * p, :],
                        rhs=et[:, 0, :],
                        start=(jb == 0), stop=(jb == NJB - 1),
                    )
                    nc.tensor.matmul(
                        out=ctx_ps[:, 1, :],
                        lhsT=v_aug[:, jb, 2 * p + 1, :],
                        rhs=et[:, 1, :],
                        start=(jb == 0), stop=(jb == NJB - 1),
                    )
                # normalize: zT rows 0:64 (head a), 64:128 (head b).
                # Only one tensor_tensor input may be PSUM -> reciprocal
                # of denominator through SBUF, then multiply.
                den = epool.tile([128, 512], f32, tag="den", bufs=1)
                nc.vector.reciprocal(
                    out=den[0:64, :], in_=ctx_ps[64:128, 0, :])
                nc.vector.reciprocal(
                    out=den[64:128, :], in_=ctx_ps[64:128, 1, :])
                nc.vector.tensor_tensor(
                    out=zT[0:64, p, bass.ts(ib, 512)],
                    in0=ctx_ps[0:64, 0, :], in1=den[0:64, :],
                    op=mybir.AluOpType.mult,
                )
                nc.vector.tensor_tensor(
                    out=zT[64:128, p, bass.ts(ib, 512)],
                    in0=ctx_ps[0:64, 1, :], in1=den[64:128, :],
                    op=mybir.AluOpType.mult,
                )

            def outproj(ib):
                # ib is a 512-wide i-block: 4x 128-row output blocks
                for sub in range(4):
                    row = ib * 4 + sub
                    for e in range(2):
                        po = psum.tile([128, 512], f32, tag="gemm")
                        for p in range(NPAIR):
                            nc.tensor.matmul(
                                out=po[:],
                                lhsT=zT[:, p, bass.ts(row, 128)],
                                rhs=wo_sb[:, p, bass.ts(e, 512)],
                                start=(p == 0), stop=(p == NPAIR - 1),
                            )
                        ot = opool.tile([128, 512], f32, tag="ot")
                        nc.vector.tensor_copy(out=ot[:], in_=po[:])
                        nc.sync.dma_start(
                            out[bass.ts(row, 128), bass.ts(e, 512)], ot[:])

            # ---- pair-major attention: pair p's projections amortize into
            # pair p-1's ACT-bound slack; outproj(ib) after attention(3,ib)
            with tc.high_priority():
                for ib in range(NSB):
                    attention(0, ib)

            # wo reuses wk's weight slot eventually; issue the DMA early
            wo_sb = wpool.tile([128, NPAIR, D], bf16, tag="w", name="wo_sb")

            for p in range(1, NPAIR):
                for sb in range(NSB):
                    proj(wk_t, xk_sb[sb], kT, bk_sb, p, sb)
                for sb in range(NSB):
                    proj(wq_t, xq_sb[sb], qT, bq_sb, p, sb)
                if p == NPAIR - 1:
                    # all wk reads emitted; wo takes wk's recycled slot
                    nc.sync.dma_start(
                        wo_sb[:], wo.rearrange("(p d) e -> d p e", d=128))
                for ib in range(NSB):
                    attention(p, ib)
                    if p == NPAIR - 1 and ib >= 1:
                        outproj(ib - 1)
            outproj(NSB - 1)

    nc.compile()
    return nc


def _prep_inputs(Q, K, V, Wq, bq, Wk, bk, Wv, bv, Wo, bo):
    """Build the 8 per-core input maps (host-side shard + transpose + cast)."""
    xt = {}  # (tensor, batch) -> [NKC,128,S] bf16
    for nm, full in (("xq", Q), ("xk", K), ("xv", V)):
        for b in range(B):
            xt[(nm, b)] = np.ascontiguousarray(full[b].T).astype(BF16)

    def w_half(W, h0):
        # W [H,T,D] -> [D, HL*T] -> [NKC,128,DL]
        w = W[h0:h0 + HL]                       # [HL,T,D]
        w = w.transpose(2, 0, 1).reshape(D, DL)  # [D, HL*T]
        return np.ascontiguousarray(w).astype(BF16)

    def b_half(bias, h0):
        return np.ascontiguousarray(
            bias[h0:h0 + HL].reshape(NPAIR, 128).T).astype(np.float32)

    in_maps = []
    for c in range(NC):
        b, half = c // 2, c % 2
        h0 = half * HL
        off = half * DL
        wo_c = np.ascontiguousarray(Wo[:, off:off + DL].T).astype(BF16)
        in_maps.append({
            "xq": xt[("xq", b)], "xk": xt[("xk", b)], "xv": xt[("xv", b)],
            "wq": w_half(Wq, h0), "wk": w_half(Wk, h0), "wv": w_half(Wv, h0),
            "bq2": b_half(bq, h0), "bk2": b_half(bk, h0),
            "wo": wo_c,
        })
    return in_maps


def _combine(results, bv, Wo, bo):
    const = (Wo.astype(np.float64) @ bv.reshape(-1).astype(np.float64)
             + bo.astype(np.float64)).astype(np.float32)
    out = np.empty((B, S, D), np.float32)
    for b in range(B):
        out[b] = results[2 * b]["out"] + results[2 * b + 1]["out"] + const
    return out


def kernel(Q, K, V, Wq, bq, Wk, bk, Wv, bv, Wo, bo, _trace=False):
    args = [np.asarray(a, np.float32) for a in
            (Q, K, V, Wq, bq, Wk, bk, Wv, bv, Wo, bo)]
    Q, K, V, Wq, bq, Wk, bk, Wv, bv, Wo, bo = args
    if "nc" not in _STATE:
        _STATE["nc"] = _build()
    nc = _STATE["nc"]
    in_maps = _prep_inputs(Q, K, V, Wq, bq, Wk, bk, Wv, bv, Wo, bo)
    try:
        res = run_bass_kernel_spmd(nc, in_maps, list(range(NC)), trace=_trace)
    except ImportError:
        res = run_bass_kernel_spmd(nc, in_maps, list(range(NC)), trace=False)
    out = _combine(res.results, bv, Wo, bo)
    if _trace:
        _STATE["last_result"] = res
    return out


# revision 23
# speedup vs baseline: 17355.9945x; 15763.8847x over previous
"""Multi-head self-attention (B=4, S=2048, D=1024, H=16) on 8 NeuronCores.

Sharding: core c handles batch b=c//2 and head-half h0=(c%2)*8 (8 of 16 heads).
Each core computes q/k/v projections for its heads, full attention, and a
partial output projection over its 512-wide slice of the concat dim.
Host sums the two partial outputs per batch and adds bo + Wo@bv (the value
bias commutes through softmax since probabilities sum to 1).

On-chip layout (all pre-transposed on host, d-major, bf16):
  qT/kT: [t, s] per head-pair stacked on partitions -> row-packed K=64
         score matmuls (two heads concurrently in PE row groups 0-1/2-3).
  scoresT: [j(key), i(query)] so softmax denom = partition-dim sum, obtained
         free via 64 ones-columns appended to v in the ctx matmul.
  exp:   one ACT op per j-tile over a 2-bank [128,2,512] PSUM AP, scale=1/8.
"""

import numpy as np
import ml_dtypes

from contextlib import ExitStack

import concourse.bacc as bacc
import concourse.bass as bass
import concourse.mybir as mybir
import concourse.tile as tile
from concourse.bass_utils import run_bass_kernel_spmd

BF16 = ml_dtypes.bfloat16

B, S, D, H, T = 4, 2048, 1024, 16, 64
HL = 8            # heads per core
DL = HL * T       # 512: local slice of concat dim
NC = 8            # cores
NPAIR = 4         # head pairs per core
NSB = 4           # 512-wide s/i blocks
NJB = 16          # 128-wide j blocks
NKC = 8           # 128-wide contraction chunks of D

f32 = mybir.dt.float32
bf16 = mybir.dt.bfloat16

_STATE = {}


def _build():
    nc = bacc.Bacc("TRN2", target_bir_lowering=False, debug=False, num_devices=NC)

    xq = nc.dram_tensor("xq", [D, S], bf16, kind="ExternalInput").ap()
    xk = nc.dram_tensor("xk", [D, S], bf16, kind="ExternalInput").ap()
    xv = nc.dram_tensor("xv", [D, S], bf16, kind="ExternalInput").ap()
    wq = nc.dram_tensor("wq", [NPAIR, 128, NKC, 128], bf16, kind="ExternalInput").ap()
    wk = nc.dram_tensor("wk", [NPAIR, 128, NKC, 128], bf16, kind="ExternalInput").ap()
    wv = nc.dram_tensor("wv", [D, DL], bf16, kind="ExternalInput").ap()
    bq2 = nc.dram_tensor("bq2", [128, NPAIR], f32, kind="ExternalInput").ap()
    bk2 = nc.dram_tensor("bk2", [128, NPAIR], f32, kind="ExternalInput").ap()
    wo = nc.dram_tensor("wo", [DL, D], bf16, kind="ExternalInput").ap()
    out = nc.dram_tensor("out", [S, D], f32, kind="ExternalOutput").ap()

    with tile.TileContext(nc) as tc:
        with ExitStack() as octx:
            # Persistent tensors
            persist = octx.enter_context(tc.tile_pool(name="persist", bufs=1))
            qT = persist.tile([128, NPAIR, S], bf16, tag="qT")
            kT = persist.tile([128, NPAIR, S], bf16, tag="kT")
            # v_aug[:, h, jb, 0:64] = v[j, t]; [..., 64:128] = 1.0
            v_aug = persist.tile([128, NJB, HL, 128], bf16, tag="v_aug")
            bq_sb = persist.tile([128, NPAIR], f32, tag="bq_sb")
            bk_sb = persist.tile([128, NPAIR], f32, tag="bk_sb")

            # whole-tensor x tiles; zT reuses xv's slot after v-proj
            xpool = octx.enter_context(tc.tile_pool(name="xp", bufs=1))
            xv_t = xpool.tile([128, NKC, S], bf16, tag="xv")
            zT = xpool.tile([128, NPAIR, S], bf16, tag="xv", name="zT")
            xspool = octx.enter_context(tc.tile_pool(name="xsp", bufs=4))
            # weights: wk/wq/wv cycle 3 slots; wo reuses wk's slot later
            wpool = octx.enter_context(tc.tile_pool(name="wp", bufs=3))
            # One PSUM pool, 8 banks total:
            #   sc   [128,2,512] x2 bufs = 4 banks (scores double-buffer)
            #   ctx  [128,2,512] x1 buf  = 2 banks
            #   gemm [128,512]   x2 bufs = 2 banks (qkv proj + out proj)
            psum = octx.enter_context(tc.tile_pool(name="psum", bufs=2,
                                                   space="PSUM"))
            epool = octx.enter_context(tc.tile_pool(name="ep", bufs=7))
            opool = octx.enter_context(tc.tile_pool(name="op", bufs=3))

            # dram views [128, chunk, cols]
            xkr = xk.rearrange("(c p) s -> p c s", p=128)
            xqr = xq.rearrange("(c p) s -> p c s", p=128)
            xvr = xv.rearrange("(c p) s -> p c s", p=128)
            wvr = wv.rearrange("(c p) d -> p c d", p=128)

            # ones columns of v_aug (memset whole region once)
            nc.vector.memset(v_aug[:, :, :, 64:128], 1.0)

            def proj(ws, xs, dstT, bias, p, sb):
                pq = psum.tile([128, 512], f32, tag="gemm", name="pq")
                for c in range(NKC):
                    nc.tensor.matmul(
                        out=pq[:],
                        lhsT=ws[:, p, c, :],
                        rhs=xs[:, c, :],
                        start=(c == 0), stop=(c == NKC - 1),
                    )
                nc.vector.tensor_scalar_add(
                    out=dstT[:, p, bass.ts(sb, 512)],
                    in0=pq[:],
                    scalar1=bias[:, p:p + 1],
                )

            def load_xsb(r, sb, tag, eng):
                t = xspool.tile([128, NKC, 512], bf16, tag=tag, name=tag)
                eng.dma_start(t[:], r[:, :, bass.ts(sb, 512)])
                return t

            # ---- critical path to first exp: pair-0 weight columns and
            # s-block-0 inputs first, split across the two DMA queues ----
            wk_t = wpool.tile([128, NPAIR, NKC, 128], bf16, tag="w",
                              name="wk_t")
            nc.gpsimd.dma_start(
                wk_t[:, 0], wk[0].rearrange("p c t -> p c t"))
            xk_sb = [load_xsb(xkr, 0, "xks", nc.gpsimd)]
            nc.gpsimd.dma_start(bk_sb[:], bk2[:, :])
            wq_t = wpool.tile([128, NPAIR, NKC, 128], bf16, tag="w",
                              name="wq_t")
            nc.sync.dma_start(
                wq_t[:, 0], wq[0].rearrange("p c t -> p c t"))
            xq_sb = [load_xsb(xqr, 0, "xqs", nc.sync)]
            nc.sync.dma_start(bq_sb[:], bq2[:, :])
            for p in range(1, NPAIR):
                nc.gpsimd.dma_start(wk_t[:, p], wk[p].rearrange("p c t -> p c t"))
                nc.sync.dma_start(wq_t[:, p], wq[p].rearrange("p c t -> p c t"))
            proj(wk_t, xk_sb[0], kT, bk_sb, 0, 0)
            proj(wq_t, xq_sb[0], qT, bq_sb, 0, 0)

            # rest of the k inputs (scores jb>=4 of the first block), then
            # v inputs on both queues, then remaining q inputs
            xk_sb.append(load_xsb(xkr, 1, "xks", nc.gpsimd))
            proj(wk_t, xk_sb[1], kT, bk_sb, 0, 1)
            wv_t = wpool.tile([128, NKC, DL], bf16, tag="w", name="wv_t")
            nc.sync.dma_start(wv_t[:], wvr[:])
            for c in range(NKC):
                eng = nc.gpsimd if c % 2 == 0 else nc.sync
                eng.dma_start(xv_t[:, c, :], xvr[:, c, :])
            xk_sb.append(load_xsb(xkr, 2, "xks", nc.gpsimd))
            xk_sb.append(load_xsb(xkr, 3, "xks", nc.gpsimd))
            for sb in range(1, NSB):
                xq_sb.append(load_xsb(xqr, sb, "xqs", nc.sync))
            proj(wk_t, xk_sb[2], kT, bk_sb, 0, 2)
            proj(wk_t, xk_sb[3], kT, bk_sb, 0, 3)
            proj(wq_t, xq_sb[1], qT, bq_sb, 0, 1)
            proj(wq_t, xq_sb[2], qT, bq_sb, 0, 2)
            proj(wq_t, xq_sb[3], qT, bq_sb, 0, 3)

            # ---- v projection (PE work before first ctx; scheduler hoists
            # independent score matmuls ahead as their inputs land) ----
            for jb in range(NJB):
                pv = psum.tile([128, HL, T], f32, tag="gemm")
                for c in range(NKC):
                    nc.tensor.matmul(
                        out=pv[:],
                        lhsT=xv_t[:, c, bass.ts(jb, 128)],
                        rhs=wv_t[:, c, :],
                        start=(c == 0), stop=(c == NKC - 1),
                    )
                nc.vector.tensor_copy(out=v_aug[:, jb, :, 0:64], in_=pv[:])

            def score_exp(p, ib, jb):
                sc = psum.tile([128, 2, 512], f32, tag="sc", name="sc")
                nc.tensor.matmul(
                    out=sc[:, 0, :],
                    lhsT=kT[0:64, p, bass.ts(jb, 128)],
                    rhs=qT[0:64, p, bass.ts(ib, 512)],
                    start=True, stop=True,
                )
                nc.tensor.matmul(
                    out=sc[:, 1, :],
                    lhsT=kT[64:128, p, bass.ts(jb, 128)],
                    rhs=qT[64:128, p, bass.ts(ib, 512)],
                    start=True, stop=True,
                )
                et = epool.tile([128, 2, 512], bf16, tag="et", bufs=7,
                                name="et")
                nc.scalar.activation(
                    out=et[:], in_=sc[:],
                    func=mybir.ActivationFunctionType.Exp,
                    scale=0.125,
                )
                return et

            def attn_head(p, ib):
                # first two j-tiles' scores+exp, emitted early so the ACT
                # pipeline stays fed across block boundaries
                return [score_exp(p, ib, 0), score_exp(p, ib, 1)]

            def attention(p, ib, head=None, next_head=None):
                next_ets = None
                ctx_ps = psum.tile([128, 2, 512], f32, tag="ctx", bufs=1)
                for jb in range(NJB):
                    if head is not None and jb < 2:
                        et = head[jb]
                    else:
                        et = score_exp(p, ib, jb)
                    if jb == NJB - 2 and next_head is not None:
                        next_ets = next_head()
                    # ctx accumulation; cols 64:128 of v_aug are ones
                    # -> rows 64:128 of ctx_ps = softmax denominator
                    nc.tensor.matmul(
                        out=ctx_ps[:, 0, :],
                        lhsT=v_aug[:, jb, 2 * p, :],
                        rhs=et[:, 0, :],
                        start=(jb == 0), stop=(jb == NJB - 1),
                    )
                    nc.tensor.matmul(
                        out=ctx_ps[:, 1, :],
                        lhsT=v_aug[:, jb, 2 * p + 1, :],
                        rhs=et[:, 1, :],
                        start=(jb == 0), stop=(jb == NJB - 1),
                    )
                # normalize: zT rows 0:64 (head a), 64:128 (head b).
                # Only one tensor_tensor input may be PSUM -> reciprocal
                # of denominator through SBUF, then multiply.
                den = epool.tile([128, 512], f32, tag="den", bufs=1)
                nc.vector.reciprocal(
                    out=den[0:64, :], in_=ctx_ps[64:128, 0, :])
                nc.vector.reciprocal(
                    out=den[64:128, :], in_=ctx_ps[64:128, 1, :])
                nc.vector.tensor_tensor(
                    out=zT[0:64, p, bass.ts(ib, 512)],
                    in0=ctx_ps[0:64, 0, :], in1=den[0:64, :],
                    op=mybir.AluOpType.mult,
                )
                nc.vector.tensor_tensor(
                    out=zT[64:128, p, bass.ts(ib, 512)],
                    in0=ctx_ps[0:64, 1, :], in1=den[64:128, :],
                    op=mybir.AluOpType.mult,
                )
                return next_ets

            def outproj(ib):
                # ib is a 512-wide i-block: 4x 128-row output blocks
                for sub in range(4):
                    row = ib * 4 + sub
                    for e in range(2):
                        po = psum.tile([128, 512], f32, tag="gemm")
                        for p in range(NPAIR):
                            nc.tensor.matmul(
                                out=po[:],
                                lhsT=zT[:, p, bass.ts(row, 128)],
                                rhs=wo_sb[:, p, bass.ts(e, 512)],
                                start=(p == 0), stop=(p == NPAIR - 1),
                            )
                        ot = opool.tile([128, 512], f32, tag="ot")
                        nc.vector.tensor_copy(out=ot[:], in_=po[:])
                        nc.sync.dma_start(
                            out[bass.ts(row, 128), bass.ts(e, 512)], ot[:])

            # ---- pair-major attention: pair p's projections amortize into
            # pair p-1's ACT-bound slack; outproj(ib) after attention(3,ib)
            with tc.high_priority():
                for ib in range(NSB):
                    attention(0, ib)

            # wo reuses wk's weight slot eventually; issue the DMA early
            wo_sb = wpool.tile([128, NPAIR, D], bf16, tag="w", name="wo_sb")

            for p in range(1, NPAIR):
                for sb in range(NSB):
                    proj(wk_t, xk_sb[sb], kT, bk_sb, p, sb)
                for sb in range(NSB):
                    proj(wq_t, xq_sb[sb], qT, bq_sb, p, sb)
                if p == NPAIR - 1:
                    # all wk reads emitted; wo takes wk's recycled slot
                    nc.sync.dma_start(
                        wo_sb[:], wo.rearrange("(p d) e -> d p e", d=128))
                for ib in range(NSB):
                    attention(p, ib)
                    if p == NPAIR - 1 and ib >= 1:
                        outproj(ib - 1)
            outproj(NSB - 1)

    nc.compile()
    return nc


def _prep_inputs(Q, K, V, Wq, bq, Wk, bk, Wv, bv, Wo, bo):
    """Build the 8 per-core input maps (host-side shard + transpose + cast)."""
    xt = {}  # (tensor, batch) -> [NKC,128,S] bf16
    for nm, full in (("xq", Q), ("xk", K), ("xv", V)):
        for b in range(B):
            xt[(nm, b)] = np.ascontiguousarray(full[b].T).astype(BF16)

    def w_half(W, h0):
        # W [H,T,D] -> [D, HL*T]
        w = W[h0:h0 + HL]                       # [HL,T,D]
        w = w.transpose(2, 0, 1).reshape(D, DL)  # [D, HL*T]
        return np.ascontiguousarray(w).astype(BF16)

    def w_half_pm(W, h0):
        # pair-major chunked: [NPAIR, 128(r), NKC(c), 128(t)]
        # value at [p, r, c, t] = wT[c*128 + r, p*128 + t]
        w = w_half(W, h0)                        # [D, DL] bf16
        w4 = w.reshape(NKC, 128, NPAIR, 128)     # [c, r, p, t]
        return np.ascontiguousarray(w4.transpose(2, 1, 0, 3))

    def b_half(bias, h0):
        return np.ascontiguousarray(
            bias[h0:h0 + HL].reshape(NPAIR, 128).T).astype(np.float32)

    in_maps = []
    for c in range(NC):
        b, half = c // 2, c % 2
        h0 = half * HL
        off = half * DL
        wo_c = np.ascontiguousarray(Wo[:, off:off + DL].T).astype(BF16)
        in_maps.append({
            "xq": xt[("xq", b)], "xk": xt[("xk", b)], "xv": xt[("xv", b)],
            "wq": w_half_pm(Wq, h0), "wk": w_half_pm(Wk, h0),
            "wv": w_half(Wv, h0),
            "bq2": b_half(bq, h0), "bk2": b_half(bk, h0),
            "wo": wo_c,
        })
    return in_maps


def _combine(results, bv, Wo, bo):
    const = (Wo.astype(np.float64) @ bv.reshape(-1).astype(np.float64)
             + bo.astype(np.float64)).astype(np.float32)
    out = np.empty((B, S, D), np.float32)
    for b in range(B):
        out[b] = results[2 * b]["out"] + results[2 * b + 1]["out"] + const
    return out


def kernel(Q, K, V, Wq, bq, Wk, bk, Wv, bv, Wo, bo, _trace=False):
    args = [np.asarray(a, np.float32) for a in
            (Q, K, V, Wq, bq, Wk, bk, Wv, bv, Wo, bo)]
    Q, K, V, Wq, bq, Wk, bk, Wv, bv, Wo, bo = args
    if "nc" not in _STATE:
        _STATE["nc"] = _build()
    nc = _STATE["nc"]
    in_maps = _prep_inputs(Q, K, V, Wq, bq, Wk, bk, Wv, bv, Wo, bo)
    try:
        res = run_bass_kernel_spmd(nc, in_maps, list(range(NC)), trace=_trace)
    except ImportError:
        res = run_bass_kernel_spmd(nc, in_maps, list(range(NC)), trace=False)
    out = _combine(res.results, bv, Wo, bo)
    if _trace:
        _STATE["last_result"] = res
    return out


# revision 27
# speedup vs baseline: 18555.8131x; 1.0691x over previous
"""Multi-head self-attention (B=4, S=2048, D=1024, H=16) on 8 NeuronCores.

Sharding: core c handles batch b=c//2 and head-half h0=(c%2)*8 (8 of 16 heads).
Each core computes q/k/v projections for its heads, full attention, and a
partial output projection over its 512-wide slice of the concat dim.
Host sums the two partial outputs per batch and adds bo + Wo@bv (the value
bias commutes through softmax since probabilities sum to 1).

On-chip layout (all pre-transposed on host, d-major, bf16):
  qT/kT: [t, s] per head-pair stacked on partitions -> row-packed K=64
         score matmuls (two heads concurrently in PE row groups 0-1/2-3).
  scoresT: [j(key), i(query)] so softmax denom = partition-dim sum, obtained
         free via 64 ones-columns appended to v in the ctx matmul.
  exp:   one ACT op per j-tile over a 2-bank [128,2,512] PSUM AP, scale=1/8.
"""

import numpy as np
import ml_dtypes

from contextlib import ExitStack

import concourse.bacc as bacc
import concourse.bass as bass
import concourse.mybir as mybir
import concourse.tile as tile
from concourse.bass_utils import run_bass_kernel_spmd

BF16 = ml_dtypes.bfloat16

B, S, D, H, T = 4, 2048, 1024, 16, 64
HL = 8            # heads per core
DL = HL * T       # 512: local slice of concat dim
NC = 8            # cores
NPAIR = 4         # head pairs per core
NSB = 4           # 512-wide s/i blocks
NJB = 16          # 128-wide j blocks
NKC = 8           # 128-wide contraction chunks of D

f32 = mybir.dt.float32
bf16 = mybir.dt.bfloat16

_STATE = {}


def _build():
    nc = bacc.Bacc("TRN2", target_bir_lowering=False, debug=False, num_devices=NC)

    xq = nc.dram_tensor("xq", [D, S], bf16, kind="ExternalInput").ap()
    xk = nc.dram_tensor("xk", [D, S], bf16, kind="ExternalInput").ap()
    xv = nc.dram_tensor("xv", [D, S], bf16, kind="ExternalInput").ap()
    wq = nc.dram_tensor("wq", [NPAIR, 128, NKC, 128], bf16, kind="ExternalInput").ap()
    wk = nc.dram_tensor("wk", [NPAIR, 128, NKC, 128], bf16, kind="ExternalInput").ap()
    wv = nc.dram_tensor("wv", [D, DL], bf16, kind="ExternalInput").ap()
    bq2 = nc.dram_tensor("bq2", [128, NPAIR], f32, kind="ExternalInput").ap()
    bk2 = nc.dram_tensor("bk2", [128, NPAIR], f32, kind="ExternalInput").ap()
    wo = nc.dram_tensor("wo", [DL, D], bf16, kind="ExternalInput").ap()
    out = nc.dram_tensor("out", [S, D], f32, kind="ExternalOutput").ap()

    with tile.TileContext(nc) as tc:
        with ExitStack() as octx:
            # Persistent tensors
            persist = octx.enter_context(tc.tile_pool(name="persist", bufs=1))
            qT = persist.tile([128, NPAIR, S], bf16, tag="qT")
            kT = persist.tile([128, NPAIR, S], bf16, tag="kT")
            # v_aug[:, h, jb, 0:64] = v[j, t]; [..., 64:128] = 1.0
            v_aug = persist.tile([128, NJB, HL, 128], bf16, tag="v_aug")
            bq_sb = persist.tile([128, NPAIR], f32, tag="bq_sb")
            bk_sb = persist.tile([128, NPAIR], f32, tag="bk_sb")

            # whole-tensor x tiles; zT reuses xv's slot after v-proj
            xpool = octx.enter_context(tc.tile_pool(name="xp", bufs=1))
            xv_t = xpool.tile([128, NKC, S], bf16, tag="xv")
            zT = xpool.tile([128, NPAIR, S], bf16, tag="xv", name="zT")
            xspool = octx.enter_context(tc.tile_pool(name="xsp", bufs=4))
            # weights: wk/wq/wv cycle 3 slots; wo reuses wk's slot later
            wpool = octx.enter_context(tc.tile_pool(name="wp", bufs=3))
            # One PSUM pool, 8 banks total:
            #   sc   [128,2,512] x2 bufs = 4 banks (scores double-buffer)
            #   ctx  [128,2,512] x1 buf  = 2 banks
            #   gemm [128,512]   x2 bufs = 2 banks (qkv proj + out proj)
            psum = octx.enter_context(tc.tile_pool(name="psum", bufs=2,
                                                   space="PSUM"))
            epool = octx.enter_context(tc.tile_pool(name="ep", bufs=7))
            opool = octx.enter_context(tc.tile_pool(name="op", bufs=3))

            # dram views [128, chunk, cols]
            xkr = xk.rearrange("(c p) s -> p c s", p=128)
            xqr = xq.rearrange("(c p) s -> p c s", p=128)
            xvr = xv.rearrange("(c p) s -> p c s", p=128)
            wvr = wv.rearrange("(c p) d -> p c d", p=128)

            # ones columns of v_aug (memset whole region once)
            nc.vector.memset(v_aug[:, :, :, 64:128], 1.0)

            def proj(ws, xs, dstT, bias, p, sb):
                pq = psum.tile([128, 512], f32, tag="gemm", name="pq")
                for c in range(NKC):
                    nc.tensor.matmul(
                        out=pq[:],
                        lhsT=ws[:, p, c, :],
                        rhs=xs[:, c, :],
                        start=(c == 0), stop=(c == NKC - 1),
                    )
                nc.vector.tensor_scalar_add(
                    out=dstT[:, p, bass.ts(sb, 512)],
                    in0=pq[:],
                    scalar1=bias[:, p:p + 1],
                )

            def load_xsb(r, sb, tag, eng):
                t = xspool.tile([128, NKC, 512], bf16, tag=tag, name=tag)
                eng.dma_start(t[:], r[:, :, bass.ts(sb, 512)])
                return t

            # ---- critical path to first exp: pair-0 weight columns and
            # s-block-0 inputs first, split across the two DMA queues ----
            wk_t = wpool.tile([128, NPAIR, NKC, 128], bf16, tag="w",
                              name="wk_t")
            nc.gpsimd.dma_start(
                wk_t[:, 0], wk[0].rearrange("p c t -> p c t"))
            xk_sb = [load_xsb(xkr, 0, "xks", nc.gpsimd)]
            nc.gpsimd.dma_start(bk_sb[:], bk2[:, :])
            wq_t = wpool.tile([128, NPAIR, NKC, 128], bf16, tag="w",
                              name="wq_t")
            nc.sync.dma_start(
                wq_t[:, 0], wq[0].rearrange("p c t -> p c t"))
            xq_sb = [load_xsb(xqr, 0, "xqs", nc.sync)]
            nc.sync.dma_start(bq_sb[:], bq2[:, :])
            for p in range(1, NPAIR):
                nc.gpsimd.dma_start(wk_t[:, p], wk[p].rearrange("p c t -> p c t"))
                nc.sync.dma_start(wq_t[:, p], wq[p].rearrange("p c t -> p c t"))
            proj(wk_t, xk_sb[0], kT, bk_sb, 0, 0)
            proj(wq_t, xq_sb[0], qT, bq_sb, 0, 0)

            # rest of the k inputs (scores jb>=4 of the first block), then
            # v inputs on both queues, then remaining q inputs
            xk_sb.append(load_xsb(xkr, 1, "xks", nc.gpsimd))
            proj(wk_t, xk_sb[1], kT, bk_sb, 0, 1)
            wv_t = wpool.tile([128, NKC, DL], bf16, tag="w", name="wv_t")
            nc.sync.dma_start(wv_t[:], wvr[:])
            for c in range(NKC):
                eng = nc.gpsimd if c % 2 == 0 else nc.sync
                eng.dma_start(xv_t[:, c, :], xvr[:, c, :])
            xk_sb.append(load_xsb(xkr, 2, "xks", nc.gpsimd))
            xk_sb.append(load_xsb(xkr, 3, "xks", nc.gpsimd))
            for sb in range(1, NSB):
                xq_sb.append(load_xsb(xqr, sb, "xqs", nc.sync))
            proj(wq_t, xq_sb[1], qT, bq_sb, 0, 1)
            proj(wk_t, xk_sb[2], kT, bk_sb, 0, 2)
            proj(wq_t, xq_sb[2], qT, bq_sb, 0, 2)
            proj(wk_t, xk_sb[3], kT, bk_sb, 0, 3)
            proj(wq_t, xq_sb[3], qT, bq_sb, 0, 3)

            # ---- v projection (PE work before first ctx; scheduler hoists
            # independent score matmuls ahead as their inputs land) ----
            for jb in range(NJB):
                pv = psum.tile([128, HL, T], f32, tag="gemm")
                for c in range(NKC):
                    nc.tensor.matmul(
                        out=pv[:],
                        lhsT=xv_t[:, c, bass.ts(jb, 128)],
                        rhs=wv_t[:, c, :],
                        start=(c == 0), stop=(c == NKC - 1),
                    )
                nc.vector.tensor_copy(out=v_aug[:, jb, :, 0:64], in_=pv[:])

            def score_exp(p, ib, jb):
                sc = psum.tile([128, 2, 512], f32, tag="sc", name="sc")
                nc.tensor.matmul(
                    out=sc[:, 0, :],
                    lhsT=kT[0:64, p, bass.ts(jb, 128)],
                    rhs=qT[0:64, p, bass.ts(ib, 512)],
                    start=True, stop=True,
                )
                nc.tensor.matmul(
                    out=sc[:, 1, :],
                    lhsT=kT[64:128, p, bass.ts(jb, 128)],
                    rhs=qT[64:128, p, bass.ts(ib, 512)],
                    start=True, stop=True,
                )
                et = epool.tile([128, 2, 512], bf16, tag="et", bufs=7,
                                name="et")
                nc.scalar.activation(
                    out=et[:], in_=sc[:],
                    func=mybir.ActivationFunctionType.Exp,
                    scale=0.125,
                )
                return et

            def attn_head(p, ib):
                # first two j-tiles' scores+exp, emitted early so the ACT
                # pipeline stays fed across block boundaries
                return [score_exp(p, ib, 0), score_exp(p, ib, 1)]

            LEAD = 5  # score/exp emission leads ctx consumption

            def attention(p, ib, head=None, next_head=None, post=None):
                next_ets = None
                ctx_ps = psum.tile([128, 2, 512], f32, tag="ctx", bufs=1)
                ets = list(head) if head else []
                produced = len(ets)
                for jb in range(NJB):
                    while produced < min(jb + LEAD, NJB):
                        ets.append(score_exp(p, ib, produced))
                        produced += 1
                        if produced == NJB and next_head is not None:
                            next_ets = next_head()
                    et = ets[jb]
                    if post is not None and jb >= 4 and post:
                        post.pop(0)()
                    # ctx accumulation; cols 64:128 of v_aug are ones
                    # -> rows 64:128 of ctx_ps = softmax denominator
                    nc.tensor.matmul(
                        out=ctx_ps[:, 0, :],
                        lhsT=v_aug[:, jb, 2 * p, :],
                        rhs=et[:, 0, :],
                        start=(jb == 0), stop=(jb == NJB - 1),
                    )
                    nc.tensor.matmul(
                        out=ctx_ps[:, 1, :],
                        lhsT=v_aug[:, jb, 2 * p + 1, :],
                        rhs=et[:, 1, :],
                        start=(jb == 0), stop=(jb == NJB - 1),
                    )
                # normalize: zT rows 0:64 (head a), 64:128 (head b).
                # Only one tensor_tensor input may be PSUM -> reciprocal
                # of denominator through SBUF, then multiply.
                den = epool.tile([128, 512], f32, tag="den", bufs=1)
                nc.vector.reciprocal(
                    out=den[0:64, :], in_=ctx_ps[64:128, 0, :])
                nc.vector.reciprocal(
                    out=den[64:128, :], in_=ctx_ps[64:128, 1, :])
                nc.vector.tensor_tensor(
                    out=zT[0:64, p, bass.ts(ib, 512)],
                    in0=ctx_ps[0:64, 0, :], in1=den[0:64, :],
                    op=mybir.AluOpType.mult,
                )
                nc.vector.tensor_tensor(
                    out=zT[64:128, p, bass.ts(ib, 512)],
                    in0=ctx_ps[0:64, 1, :], in1=den[64:128, :],
                    op=mybir.AluOpType.mult,
                )
                return next_ets

            def outproj_piece(row, e):
                po = psum.tile([128, 512], f32, tag="gemm", name="po")
                for p in range(NPAIR):
                    nc.tensor.matmul(
                        out=po[:],
                        lhsT=zT[:, p, bass.ts(row, 128)],
                        rhs=wo_sb[:, p, bass.ts(e, 512)],
                        start=(p == 0), stop=(p == NPAIR - 1),
                    )
                ot = opool.tile([128, 512], f32, tag="ot")
                nc.vector.tensor_copy(out=ot[:], in_=po[:])
                nc.sync.dma_start(
                    out[bass.ts(row, 128), bass.ts(e, 512)], ot[:])

            def outproj_pieces(ib):
                return [
                    (lambda row=ib * 4 + sub, e=e: outproj_piece(row, e))
                    for sub in range(4) for e in range(2)
                ]

            def outproj(ib):
                # ib is a 512-wide i-block: 4x 128-row output blocks
                for sub in range(4):
                    row = ib * 4 + sub
                    for e in range(2):
                        po = psum.tile([128, 512], f32, tag="gemm")
                        for p in range(NPAIR):
                            nc.tensor.matmul(
                                out=po[:],
                                lhsT=zT[:, p, bass.ts(row, 128)],
                                rhs=wo_sb[:, p, bass.ts(e, 512)],
                                start=(p == 0), stop=(p == NPAIR - 1),
                            )
                        ot = opool.tile([128, 512], f32, tag="ot")
                        nc.vector.tensor_copy(out=ot[:], in_=po[:])
                        nc.sync.dma_start(
                            out[bass.ts(row, 128), bass.ts(e, 512)], ot[:])

            # ---- pair-major attention: pair p's projections amortize into
            # pair p-1's ACT-bound slack; outproj(ib) after attention(3,ib)
            with tc.high_priority():
                ets = None
                for ib in range(NSB):
                    nh = (lambda ib=ib: attn_head(0, ib + 1)) \
                        if ib < NSB - 1 else None
                    ets = attention(0, ib, head=ets, next_head=nh)

            # wo reuses wk's weight slot eventually; issue the DMA early
            wo_sb = wpool.tile([128, NPAIR, D], bf16, tag="w", name="wo_sb")

            for p in range(1, NPAIR):
                for sb in range(NSB):
                    proj(wk_t, xk_sb[sb], kT, bk_sb, p, sb)
                for sb in range(NSB):
                    proj(wq_t, xq_sb[sb], qT, bq_sb, p, sb)
                if p == NPAIR - 1:
                    # all wk reads emitted; wo takes wk's recycled slot
                    nc.sync.dma_start(
                        wo_sb[:], wo.rearrange("(p d) e -> d p e", d=128))
                ets = None
                for ib in range(NSB):
                    nh = (lambda p=p, ib=ib: attn_head(p, ib + 1)) \
                        if ib < NSB - 1 else None
                    post = (outproj_pieces(ib - 1)
                            if p == NPAIR - 1 and ib >= 1 else None)
                    ets = attention(p, ib, head=ets, next_head=nh, post=post)
            outproj(NSB - 1)

    nc.compile()
    return nc


def _prep_inputs(Q, K, V, Wq, bq, Wk, bk, Wv, bv, Wo, bo):
    """Build the 8 per-core input maps (host-side shard + transpose + cast)."""
    xt = {}  # (tensor, batch) -> [NKC,128,S] bf16
    for nm, full in (("xq", Q), ("xk", K), ("xv", V)):
        for b in range(B):
            xt[(nm, b)] = np.ascontiguousarray(full[b].T).astype(BF16)

    def w_half(W, h0):
        # W [H,T,D] -> [D, HL*T]
        w = W[h0:h0 + HL]                       # [HL,T,D]
        w = w.transpose(2, 0, 1).reshape(D, DL)  # [D, HL*T]
        return np.ascontiguousarray(w).astype(BF16)

    def w_half_pm(W, h0):
        # pair-major chunked: [NPAIR, 128(r), NKC(c), 128(t)]
        # value at [p, r, c, t] = wT[c*128 + r, p*128 + t]
        w = w_half(W, h0)                        # [D, DL] bf16
        w4 = w.reshape(NKC, 128, NPAIR, 128)     # [c, r, p, t]
        return np.ascontiguousarray(w4.transpose(2, 1, 0, 3))

    def b_half(bias, h0):
        return np.ascontiguousarray(
            bias[h0:h0 + HL].reshape(NPAIR, 128).T).astype(np.float32)

    in_maps = []
    for c in range(NC):
        b, half = c // 2, c % 2
        h0 = half * HL
        off = half * DL
        wo_c = np.ascontiguousarray(Wo[:, off:off + DL].T).astype(BF16)
        in_maps.append({
            "xq": xt[("xq", b)], "xk": xt[("xk", b)], "xv": xt[("xv", b)],
            "wq": w_half_pm(Wq, h0), "wk": w_half_pm(Wk, h0),
            "wv": w_half(Wv, h0),
            "bq2": b_half(bq, h0), "bk2": b_half(bk, h0),
            "wo": wo_c,
        })
    return in_maps


def _combine(results, bv, Wo, bo):
    const = (Wo.astype(np.float64) @ bv.reshape(-1).astype(np.float64)
             + bo.astype(np.float64)).astype(np.float32)
    out = np.empty((B, S, D), np.float32)
    for b in range(B):
        out[b] = results[2 * b]["out"] + results[2 * b + 1]["out"] + const
    return out


def kernel(Q, K, V, Wq, bq, Wk, bk, Wv, bv, Wo, bo, _trace=False):
    args = [np.asarray(a, np.float32) for a in
            (Q, K, V, Wq, bq, Wk, bk, Wv, bv, Wo, bo)]
    Q, K, V, Wq, bq, Wk, bk, Wv, bv, Wo, bo = args
    if "nc" not in _STATE:
        _STATE["nc"] = _build()
    nc = _STATE["nc"]
    in_maps = _prep_inputs(Q, K, V, Wq, bq, Wk, bk, Wv, bv, Wo, bo)
    try:
        res = run_bass_kernel_spmd(nc, in_maps, list(range(NC)), trace=_trace)
    except ImportError:
        res = run_bass_kernel_spmd(nc, in_maps, list(range(NC)), trace=False)
    out = _combine(res.results, bv, Wo, bo)
    if _trace:
        _STATE["last_result"] = res
    return out
